# revision 12
# baseline (speedup 1.0000x reference)
"""Trainium2 Bass kernel for nn_MultiHeadAttention (softmax over QUERY axis).

Strategy: data-parallel over batch (B=8 -> one batch element per NeuronCore).
Per core (all layouts chosen so every DMA is contiguous and the softmax
reduction runs along the free axis):

  inputs (host pre-transposed):
    xqT/xkT/xvT [D, L] f32, maskbT [L(k), L(q)] bf16 (= (mask-1)*1e9 transposed)
    Wq/Wk/Wv [D, H*DK] f32, Wf [H*DV, D] f32, biases repacked

  QT = Wq^T @ q^T   [HDK, L]  (bf16 stored)      lhsT=Wq chunk, rhs=xqT chunk
  KT = Wk^T @ k^T   [HDK, L]  (bf16 stored)
  V  = v @ Wv       [L, HDV]  (f32)              lhsT=xvT chunk, rhs=Wv chunk
  per head h:
    S_T[k,q] = K_h @ Q_h^T + maskb   (PSUM: identity-matmul copies mask bias
                                      with start=True, QK^T accumulates on top)
    e = exp(S_T/8)  on ScalarE with fused accum_out -> s[k] = sum_q e[k,q]
    attn_T = e * (1/s)   (VectorE tensor_scalar, per-partition scalar)
    out_T[d,q] += V_h^T-as-lhsT @ attn_T         (accumulate over k tiles)
  outT = Wf^T @ aoT + bf   [D, L]

  outputs: attn_t [H, L(k), L(q)] f32, outT [D, L] f32 -- host returns
  transposed views to match the reference layout.
"""

import sys

if "/opt/trn_rl_repo" not in sys.path:
    sys.path.insert(0, "/opt/trn_rl_repo")

import numpy as np
import ml_dtypes

B, L, D = 8, 1024, 1024
H, DK, DV = 16, 64, 64
P = 128
NKT = L // P      # 8 k-tiles
NFT = D // P      # 8 feature/contraction tiles
QC = 512          # matmul moving free-dim chunk (1 PSUM bank of fp32)
NQ = L // QC      # 2

_CACHE = {}


def _build_nc():
    import concourse.bacc as bacc
    import concourse.mybir as mybir
    import concourse.tile as tile
    from concourse.masks import make_identity
    from contextlib import ExitStack

    f32 = mybir.dt.float32
    bf16 = mybir.dt.bfloat16
    r32 = mybir.dt.float32r
    EXP = mybir.ActivationFunctionType.Exp

    nc = bacc.Bacc(None, target_bir_lowering=False, debug=False)

    xqT = nc.dram_tensor("xqT", [D, L], r32, kind="ExternalInput")
    xkT = nc.dram_tensor("xkT", [D, L], r32, kind="ExternalInput")
    xvT = nc.dram_tensor("xvT", [D, L], r32, kind="ExternalInput")
    maskbT = nc.dram_tensor("maskbT", [L, L], bf16, kind="ExternalInput")
    Wq_d = nc.dram_tensor("Wq", [D, H * DK], r32, kind="ExternalInput")
    Wk_d = nc.dram_tensor("Wk", [D, H * DK], r32, kind="ExternalInput")
    Wv_d = nc.dram_tensor("Wv", [D, H * DV], r32, kind="ExternalInput")
    Wf_d = nc.dram_tensor("Wf", [H * DV, D], r32, kind="ExternalInput")
    bq_d = nc.dram_tensor("bq", [P, NFT], f32, kind="ExternalInput")
    bk_d = nc.dram_tensor("bk", [P, NFT], f32, kind="ExternalInput")
    bv_d = nc.dram_tensor("bv", [1, H * DV], bf16, kind="ExternalInput")
    bf_d = nc.dram_tensor("bfb", [P, NFT], f32, kind="ExternalInput")
    attn_t = nc.dram_tensor("attn_t", [H, L, L], f32, kind="ExternalOutput")
    outT = nc.dram_tensor("outT", [D, L], f32, kind="ExternalOutput")

    with tile.TileContext(nc) as tc, ExitStack() as top:
        const = top.enter_context(tc.tile_pool(name="const", bufs=1))
        ident = const.tile([P, P], bf16)
        make_identity(nc, ident[:])
        ones_row = const.tile([1, P], bf16)
        nc.gpsimd.memset(ones_row[:], 1.0)
        bq_sb = const.tile([P, NFT], f32)
        bk_sb = const.tile([P, NFT], f32)
        bf_sb = const.tile([P, NFT], f32)
        bv_sb = const.tile([1, H * DV], bf16)
        nc.sync.dma_start(bq_sb[:], bq_d[:, :])
        nc.sync.dma_start(bk_sb[:], bk_d[:, :])
        nc.sync.dma_start(bf_sb[:], bf_d[:, :])
        nc.sync.dma_start(bv_sb[:], bv_d[:, :])

        maskp = top.enter_context(tc.tile_pool(name="maskp", bufs=1))
        mask_sb = maskp.tile([P, NKT, L], bf16)
        nc.scalar.dma_start(mask_sb[:], maskbT[:, :].rearrange("(i p) q -> p i q", p=P))

        qkv = top.enter_context(tc.tile_pool(name="qkv", bufs=1))
        QT_sb = qkv.tile([P, NFT, L], bf16)
        KT_sb = qkv.tile([P, NFT, L], bf16)
        V_sb = qkv.tile([P, NKT, L], r32)
        aoT_sb = qkv.tile([P, NFT, L], r32)

        # ---------------- phase 1: projections (V first: attention pair 0
        # depends on V fully + QT/KT f-tile 0, so emit its inputs earliest) --
        with ExitStack() as ph1:
            xin = ph1.enter_context(tc.tile_pool(name="xin", bufs=2))

            with ExitStack() as vph:
                wvst = vph.enter_context(tc.tile_pool(name="wvst", bufs=3))
                vpsum = vph.enter_context(
                    tc.tile_pool(name="vpsum", bufs=1, space="PSUM")
                )
                xv_sb = xin.tile([P, NFT, L], r32, tag="xin", name="xv_sb")
                xvr = xvT[:, :].rearrange("(i p) t -> p i t", p=P)
                for ic in range(NFT):
                    nc.scalar.dma_start(xv_sb[:, ic, :], xvr[:, ic, :])
                for j in range(NQ):  # halves of the HDV feature range
                    sl = slice(j * QC, (j + 1) * QC)
                    pv = [
                        vpsum.tile([P, QC], f32, tag=f"vps{tt}", name=f"pv{tt}")
                        for tt in range(NKT)
                    ]
                    for ic in range(NFT):
                        wv_t = wvst.tile([P, QC], r32, tag="wvst", name="wv_t")
                        nc.sync.dma_start(wv_t[:], Wv_d[ic * P : (ic + 1) * P, sl])
                        for tt in range(NKT):
                            nc.tensor.matmul(
                                pv[tt][:, :],
                                lhsT=xv_sb[:, ic, tt * P : (tt + 1) * P],
                                rhs=wv_t[:],
                                start=(ic == 0),
                                stop=False,
                            )
                    for tt in range(NKT):
                        # bias add via rank-1 matmul: out[t, f] += 1 * bv[f]
                        nc.tensor.matmul(
                            pv[tt][:, :],
                            lhsT=ones_row[:],
                            rhs=bv_sb[0:1, sl],
                            start=False,
                            stop=True,
                        )
                        nc.vector.tensor_copy(V_sb[:, tt, sl], pv[tt][:, :])

            with ExitStack() as qk:
                wst = qk.enter_context(tc.tile_pool(name="wst", bufs=3))
                ppsum = qk.enter_context(
                    tc.tile_pool(name="ppsum", bufs=1, space="PSUM")
                )
                for xdram, wdram, bias_sb, out_sb in (
                    (xkT, Wk_d, bk_sb, KT_sb),
                    (xqT, Wq_d, bq_sb, QT_sb),
                ):
                    x_sb = xin.tile([P, NFT, L], r32, tag="xin", name="x_sb")
                    xr = xdram[:, :].rearrange("(i p) t -> p i t", p=P)
                    for ic in range(NFT):
                        nc.scalar.dma_start(x_sb[:, ic, :], xr[:, ic, :])
                    for fh in range(2):  # halves of the feature tiles
                        fts = range(fh * NFT // 2, (fh + 1) * NFT // 2)
                        ps = {
                            ft: ppsum.tile([P, L], f32, tag=f"pps{ft % 4}", name=f"ps_qk{ft}")
                            for ft in fts
                        }
                        for ic in range(NFT):
                            w_t = wst.tile([P, D], r32, tag="wst", name="w_t")
                            nc.sync.dma_start(
                                w_t[:], wdram[ic * P : (ic + 1) * P, :]
                            )
                            for ft in fts:
                                for j in range(NQ):
                                    sl = slice(j * QC, (j + 1) * QC)
                                    nc.tensor.matmul(
                                        ps[ft][:, sl],
                                        lhsT=w_t[:, ft * P : (ft + 1) * P],
                                        rhs=x_sb[:, ic, sl],
                                        start=(ic == 0),
                                        stop=(ic == NFT - 1),
                                    )
                        for ft in fts:
                            nc.vector.tensor_scalar_add(
                                out_sb[:, ft, :], ps[ft][:, :], bias_sb[:, ft : ft + 1]
                            )

        # ---------------- phase 2: attention ----------------
        wfp = top.enter_context(tc.tile_pool(name="wfp", bufs=1))
        Wf_sb = wfp.tile([P, NFT, D], r32)
        wfr = Wf_d[:, :].rearrange("(i p) d -> p i d", p=P)
        for ic in range(NFT):
            nc.scalar.dma_start(Wf_sb[:, ic, :], wfr[:, ic, :])
        with ExitStack() as ph2:
            spsum = ph2.enter_context(tc.tile_pool(name="spsum", bufs=2, space="PSUM"))
            opsum = ph2.enter_context(tc.tile_pool(name="opsum", bufs=2, space="PSUM"))
            epool = ph2.enter_context(tc.tile_pool(name="epool", bufs=3))
            apool = ph2.enter_context(tc.tile_pool(name="apool", bufs=7))
            stat = ph2.enter_context(tc.tile_pool(name="stat", bufs=8))

            AV_LAG = 2  # defer AV matmuls so PE never stalls on the DVE normalize

            for hp in range(H // 2):
                heads = (2 * hp, 2 * hp + 1)
                po = {h: opsum.tile([64, L], f32, tag="po", name=f"po{h}") for h in heads}
                a_ts = {}

                def emit_av(i):
                    for j in range(NQ):
                        sl = slice(j * QC, (j + 1) * QC)
                        for h in heads:
                            nc.tensor.matmul(
                                po[h][:, sl],
                                lhsT=V_sb[:, i, h * DV : (h + 1) * DV],
                                rhs=a_ts[(h, i)][:, sl],
                                start=(i == 0),
                                stop=(i == NKT - 1),
                            )

                for i in range(NKT + AV_LAG):
                    if i < NKT:
                        pss = {}
                        for h in heads:
                            pss[h] = spsum.tile([P, L], f32, tag="ps", name=f"ps{h}_{i}")
                            for j in range(NQ):
                                sl = slice(j * QC, (j + 1) * QC)
                                nc.tensor.matmul(
                                    pss[h][:, sl],
                                    lhsT=ident[:],
                                    rhs=mask_sb[:, i, sl],
                                    start=True,
                                    stop=False,
                                )
                        # S matmuls: alternate heads so adjacent instructions
                        # use disjoint PE row groups (A: rows 0-63, B: 64-127);
                        # critical section keeps the scheduler from regrouping
                        with tc.tile_critical():
                            for j in range(NQ):
                                sl = slice(j * QC, (j + 1) * QC)
                                for h in heads:
                                    hi, ho = h // 2, 64 * (h % 2)
                                    nc.tensor.matmul(
                                        pss[h][:, sl],
                                        lhsT=KT_sb[ho : ho + 64, hi, i * P : (i + 1) * P],
                                        rhs=QT_sb[ho : ho + 64, hi, sl],
                                        start=False,
                                        stop=True,
                                    )
                        for h in heads:
                            e_t = epool.tile([P, L], f32, tag="e", name=f"e{h}_{i}")
                            s_t = stat.tile([P, 1], f32, tag="s", name=f"s{h}_{i}")
                            nc.scalar.activation(
                                e_t[:], pss[h][:, :], EXP, scale=0.125, accum_out=s_t[:]
                            )
                            r_t = stat.tile([P, 1], f32, tag="r", name=f"r{h}_{i}")
                            nc.vector.reciprocal(r_t[:], s_t[:])
                            a_t = apool.tile([P, L], r32, tag="a", name=f"a{h}_{i}")
                            nc.vector.tensor_scalar_mul(a_t[:], e_t[:], r_t[:])
                            nc.sync.dma_start(
                                attn_t[h, i * P : (i + 1) * P, :], a_t[:].bitcast(f32)
                            )
                            a_ts[(h, i)] = a_t
                    if i >= AV_LAG:
                        emit_av(i - AV_LAG)
                for h in heads:
                    ho = 64 * (h % 2)
                    nc.vector.tensor_copy(aoT_sb[ho : ho + 64, h // 2, :], po[h][:, :])

        # ---------------- phase 3: output fc ----------------
        with ExitStack() as ph3:
            fcpsum = ph3.enter_context(
                tc.tile_pool(name="fcpsum", bufs=2, space="PSUM")
            )
            fco = ph3.enter_context(tc.tile_pool(name="fco", bufs=3))
            for dt in range(NFT):
                ps = fcpsum.tile([P, L], f32, tag="fc", name=f"fc{dt}")
                for j in range(NQ):
                    sl = slice(j * QC, (j + 1) * QC)
                    for ic in range(NFT):
                        nc.tensor.matmul(
                            ps[:, sl],
                            lhsT=Wf_sb[:, ic, dt * P : (dt + 1) * P],
                            rhs=aoT_sb[:, ic, sl],
                            start=(ic == 0),
                            stop=(ic == NFT - 1),
                        )
                o_t = fco.tile([P, L], f32, tag="fco", name=f"o{dt}")
                nc.vector.tensor_scalar_add(o_t[:], ps[:], bf_sb[:, dt : dt + 1])
                nc.sync.dma_start(outT[dt * P : (dt + 1) * P, :], o_t[:])

    nc.compile()
    return nc


def get_nc():
    if "nc" not in _CACHE:
        _CACHE["nc"] = _build_nc()
    return _CACHE["nc"]


def make_in_maps(q, k, v, mask, Wq, bq, Wk, bk, Wv, bv, Wf, bf):
    bf16 = ml_dtypes.bfloat16
    f32 = np.float32
    q = np.asarray(q, f32)
    k = np.asarray(k, f32)
    v = np.asarray(v, f32)
    mask = np.asarray(mask)
    Wq_, Wk_, Wv_, Wf_ = (np.ascontiguousarray(np.asarray(w, f32)) for w in (Wq, Wk, Wv, Wf))
    bq_ = np.ascontiguousarray(np.asarray(bq, f32).reshape(NFT, P).T)
    bk_ = np.ascontiguousarray(np.asarray(bk, f32).reshape(NFT, P).T)
    bf_ = np.ascontiguousarray(np.asarray(bf, f32).reshape(NFT, P).T)
    bv_ = np.asarray(bv, f32).reshape(1, H * DV).astype(bf16)
    in_maps = []
    for b in range(B):
        maskb = ((np.asarray(mask[b, 0], f32) - 1.0) * 1e9).T
        in_maps.append(
            {
                "xqT": np.ascontiguousarray(q[b].T),
                "xkT": np.ascontiguousarray(k[b].T),
                "xvT": np.ascontiguousarray(v[b].T),
                "maskbT": np.ascontiguousarray(maskb).astype(bf16),
                "Wq": Wq_,
                "Wk": Wk_,
                "Wv": Wv_,
                "Wf": Wf_,
                "bq": bq_,
                "bk": bk_,
                "bv": bv_,
                "bfb": bf_,
            }
        )
    return in_maps


def assemble(results):
    out = np.stack([np.asarray(results[b]["outT"]).T for b in range(B)])
    attn = np.stack(
        [np.asarray(results[b]["attn_t"]).transpose(0, 2, 1) for b in range(B)]
    )
    return out, attn


def kernel(q, k, v, mask, Wq, bq, Wk, bk, Wv, bv, Wf, bf):
    from concourse.bass_utils import run_bass_kernel_spmd

    nc = get_nc()
    in_maps = make_in_maps(q, k, v, mask, Wq, bq, Wk, bk, Wv, bv, Wf, bf)
    res = run_bass_kernel_spmd(nc, in_maps, list(range(B))).results
    return assemble(res)


if __name__ == "__main__":
    nc = get_nc()
    print("built ok")


# revision 13
# speedup vs baseline: 1.7685x; 1.7685x over previous
"""Trainium2 Bass kernel for nn_MultiHeadAttention (softmax over QUERY axis).

Strategy: data-parallel over batch (B=8 -> one batch element per NeuronCore).
Per core (all layouts chosen so every DMA is contiguous and the softmax
reduction runs along the free axis):

  inputs (host pre-transposed):
    xqT/xkT/xvT [D, L] f32, maskbT [L(k), L(q)] bf16 (= (mask-1)*1e9 transposed)
    Wq/Wk/Wv [D, H*DK] f32, Wf [H*DV, D] f32, biases repacked

  QT = Wq^T @ q^T   [HDK, L]  (bf16 stored)      lhsT=Wq chunk, rhs=xqT chunk
  KT = Wk^T @ k^T   [HDK, L]  (bf16 stored)
  V  = v @ Wv       [L, HDV]  (f32)              lhsT=xvT chunk, rhs=Wv chunk
  per head h:
    S_T[k,q] = K_h @ Q_h^T + maskb   (PSUM: identity-matmul copies mask bias
                                      with start=True, QK^T accumulates on top)
    e = exp(S_T/8)  on ScalarE with fused accum_out -> s[k] = sum_q e[k,q]
    attn_T = e * (1/s)   (VectorE tensor_scalar, per-partition scalar)
    out_T[d,q] += V_h^T-as-lhsT @ attn_T         (accumulate over k tiles)
  outT = Wf^T @ aoT + bf   [D, L]

  outputs: attn_t [H, L(k), L(q)] f32, outT [D, L] f32 -- host returns
  transposed views to match the reference layout.
"""

import sys

if "/opt/trn_rl_repo" not in sys.path:
    sys.path.insert(0, "/opt/trn_rl_repo")

import numpy as np
import ml_dtypes

B, L, D = 8, 1024, 1024
H, DK, DV = 16, 64, 64
P = 128
NKT = L // P      # 8 k-tiles
NFT = D // P      # 8 feature/contraction tiles
QC = 512          # matmul moving free-dim chunk (1 PSUM bank of fp32)
NQ = L // QC      # 2

_CACHE = {}


def _build_nc():
    import concourse.bacc as bacc
    import concourse.mybir as mybir
    import concourse.tile as tile
    from concourse.masks import make_identity
    from contextlib import ExitStack

    f32 = mybir.dt.float32
    bf16 = mybir.dt.bfloat16
    r32 = mybir.dt.float32r
    EXP = mybir.ActivationFunctionType.Exp

    nc = bacc.Bacc(None, target_bir_lowering=False, debug=False)

    xqT = nc.dram_tensor("xqT", [D, L], r32, kind="ExternalInput")
    xkT = nc.dram_tensor("xkT", [D, L], r32, kind="ExternalInput")
    xvT = nc.dram_tensor("xvT", [D, L], r32, kind="ExternalInput")
    maskbT = nc.dram_tensor("maskbT", [L, L], bf16, kind="ExternalInput")
    Wq_d = nc.dram_tensor("Wq", [D, H * DK], r32, kind="ExternalInput")
    Wk_d = nc.dram_tensor("Wk", [D, H * DK], r32, kind="ExternalInput")
    Wv_d = nc.dram_tensor("Wv", [D, H * DV], r32, kind="ExternalInput")
    Wf_d = nc.dram_tensor("Wf", [H * DV, D], r32, kind="ExternalInput")
    bq_d = nc.dram_tensor("bq", [P, NFT], f32, kind="ExternalInput")
    bk_d = nc.dram_tensor("bk", [P, NFT], f32, kind="ExternalInput")
    bv_d = nc.dram_tensor("bv", [1, H * DV], bf16, kind="ExternalInput")
    bf_d = nc.dram_tensor("bfb", [P, NFT], f32, kind="ExternalInput")
    attn_t = nc.dram_tensor("attn_t", [H, L, L], f32, kind="ExternalOutput")
    outT = nc.dram_tensor("outT", [D, L], f32, kind="ExternalOutput")

    with tile.TileContext(nc) as tc, ExitStack() as top:
        const = top.enter_context(tc.tile_pool(name="const", bufs=1))
        ident = const.tile([P, P], bf16)
        make_identity(nc, ident[:])
        ones_row = const.tile([1, P], bf16)
        nc.gpsimd.memset(ones_row[:], 1.0)
        bq_sb = const.tile([P, NFT], f32)
        bk_sb = const.tile([P, NFT], f32)
        bf_sb = const.tile([P, NFT], f32)
        bv_sb = const.tile([1, H * DV], bf16)
        nc.sync.dma_start(bq_sb[:], bq_d[:, :])
        nc.sync.dma_start(bk_sb[:], bk_d[:, :])
        nc.sync.dma_start(bf_sb[:], bf_d[:, :])
        nc.sync.dma_start(bv_sb[:], bv_d[:, :])

        maskp = top.enter_context(tc.tile_pool(name="maskp", bufs=1))
        mask_sb = maskp.tile([P, NKT, L], bf16)
        nc.scalar.dma_start(mask_sb[:], maskbT[:, :].rearrange("(i p) q -> p i q", p=P))

        qkv = top.enter_context(tc.tile_pool(name="qkv", bufs=1))
        QT_sb = qkv.tile([P, NFT, L], bf16)
        KT_sb = qkv.tile([P, NFT, L], bf16)
        V_sb = qkv.tile([P, NKT, L], r32)
        aoT_sb = qkv.tile([P, NFT, L], r32)

        # ---------------- phase 1: projections (V first: attention pair 0
        # depends on V fully + QT/KT f-tile 0, so emit its inputs earliest) --
        with ExitStack() as ph1:
            xin = ph1.enter_context(tc.tile_pool(name="xin", bufs=2))

            with ExitStack() as vph:
                wvst = vph.enter_context(tc.tile_pool(name="wvst", bufs=3))
                vpsum = vph.enter_context(
                    tc.tile_pool(name="vpsum", bufs=1, space="PSUM")
                )
                xv_sb = xin.tile([P, NFT, L], r32, tag="xin", name="xv_sb")
                xvr = xvT[:, :].rearrange("(i p) t -> p i t", p=P)
                for ic in range(NFT):
                    nc.scalar.dma_start(xv_sb[:, ic, :], xvr[:, ic, :])
                for j in range(NQ):  # halves of the HDV feature range
                    sl = slice(j * QC, (j + 1) * QC)
                    pv = [
                        vpsum.tile([P, QC], f32, tag=f"vps{tt}", name=f"pv{tt}")
                        for tt in range(NKT)
                    ]
                    for ic in range(NFT):
                        wv_t = wvst.tile([P, QC], r32, tag="wvst", name="wv_t")
                        nc.sync.dma_start(wv_t[:], Wv_d[ic * P : (ic + 1) * P, sl])
                        for tt in range(NKT):
                            nc.tensor.matmul(
                                pv[tt][:, :],
                                lhsT=xv_sb[:, ic, tt * P : (tt + 1) * P],
                                rhs=wv_t[:],
                                start=(ic == 0),
                                stop=False,
                            )
                    for tt in range(NKT):
                        # bias add via rank-1 matmul: out[t, f] += 1 * bv[f]
                        nc.tensor.matmul(
                            pv[tt][:, :],
                            lhsT=ones_row[:],
                            rhs=bv_sb[0:1, sl],
                            start=False,
                            stop=True,
                        )
                        nc.vector.tensor_copy(V_sb[:, tt, sl], pv[tt][:, :])

            with ExitStack() as qk:
                wst = qk.enter_context(tc.tile_pool(name="wst", bufs=3))
                ppsum = qk.enter_context(
                    tc.tile_pool(name="ppsum", bufs=1, space="PSUM")
                )
                for xdram, wdram, bias_sb, out_sb in (
                    (xkT, Wk_d, bk_sb, KT_sb),
                    (xqT, Wq_d, bq_sb, QT_sb),
                ):
                    x_sb = xin.tile([P, NFT, L], r32, tag="xin", name="x_sb")
                    xr = xdram[:, :].rearrange("(i p) t -> p i t", p=P)
                    for ic in range(NFT):
                        nc.scalar.dma_start(x_sb[:, ic, :], xr[:, ic, :])
                    for fh in range(2):  # halves of the feature tiles
                        fts = range(fh * NFT // 2, (fh + 1) * NFT // 2)
                        ps = {
                            ft: ppsum.tile([P, L], f32, tag=f"pps{ft % 4}", name=f"ps_qk{ft}")
                            for ft in fts
                        }
                        for ic in range(NFT):
                            w_t = wst.tile([P, D], r32, tag="wst", name="w_t")
                            nc.sync.dma_start(
                                w_t[:], wdram[ic * P : (ic + 1) * P, :]
                            )
                            for ft in fts:
                                for j in range(NQ):
                                    sl = slice(j * QC, (j + 1) * QC)
                                    nc.tensor.matmul(
                                        ps[ft][:, sl],
                                        lhsT=w_t[:, ft * P : (ft + 1) * P],
                                        rhs=x_sb[:, ic, sl],
                                        start=(ic == 0),
                                        stop=(ic == NFT - 1),
                                    )
                        for ft in fts:
                            nc.vector.tensor_scalar_add(
                                out_sb[:, ft, :], ps[ft][:, :], bias_sb[:, ft : ft + 1]
                            )

        # ---------------- phase 2: attention ----------------
        wfp = top.enter_context(tc.tile_pool(name="wfp", bufs=1))
        Wf_sb = wfp.tile([P, NFT, D], r32)
        wfr = Wf_d[:, :].rearrange("(i p) d -> p i d", p=P)
        for ic in range(NFT):
            nc.scalar.dma_start(Wf_sb[:, ic, :], wfr[:, ic, :])
        with ExitStack() as ph2:
            spsum = ph2.enter_context(tc.tile_pool(name="spsum", bufs=2, space="PSUM"))
            opsum = ph2.enter_context(tc.tile_pool(name="opsum", bufs=2, space="PSUM"))
            epool = ph2.enter_context(tc.tile_pool(name="epool", bufs=3))
            apool = ph2.enter_context(tc.tile_pool(name="apool", bufs=7))
            stat = ph2.enter_context(tc.tile_pool(name="stat", bufs=8))

            AV_LAG = 2  # defer AV matmuls so PE never stalls on the DVE normalize

            for hp in range(H // 2):
                heads = (2 * hp, 2 * hp + 1)
                po = {h: opsum.tile([64, L], f32, tag="po", name=f"po{h}") for h in heads}
                a_ts = {}

                def emit_av(i):
                    for j in range(NQ):
                        sl = slice(j * QC, (j + 1) * QC)
                        for h in heads:
                            nc.tensor.matmul(
                                po[h][:, sl],
                                lhsT=V_sb[:, i, h * DV : (h + 1) * DV],
                                rhs=a_ts[(h, i)][:, sl],
                                start=(i == 0),
                                stop=(i == NKT - 1),
                            )

                for i in range(NKT + AV_LAG):
                    if i < NKT:
                        pss = {}
                        for h in heads:
                            pss[h] = spsum.tile([P, L], f32, tag="ps", name=f"ps{h}_{i}")
                            for j in range(NQ):
                                sl = slice(j * QC, (j + 1) * QC)
                                nc.tensor.matmul(
                                    pss[h][:, sl],
                                    lhsT=ident[:],
                                    rhs=mask_sb[:, i, sl],
                                    start=True,
                                    stop=False,
                                )
                        # S matmuls: alternate heads so adjacent instructions
                        # use disjoint PE row groups (A: rows 0-63, B: 64-127)
                        for j in range(NQ):
                            sl = slice(j * QC, (j + 1) * QC)
                            for h in heads:
                                hi, ho = h // 2, 64 * (h % 2)
                                nc.tensor.matmul(
                                    pss[h][:, sl],
                                    lhsT=KT_sb[ho : ho + 64, hi, i * P : (i + 1) * P],
                                    rhs=QT_sb[ho : ho + 64, hi, sl],
                                    start=False,
                                    stop=True,
                                )
                        for h in heads:
                            e_t = epool.tile([P, L], f32, tag="e", name=f"e{h}_{i}")
                            s_t = stat.tile([P, 1], f32, tag="s", name=f"s{h}_{i}")
                            nc.scalar.activation(
                                e_t[:], pss[h][:, :], EXP, scale=0.125, accum_out=s_t[:]
                            )
                            r_t = stat.tile([P, 1], f32, tag="r", name=f"r{h}_{i}")
                            nc.vector.reciprocal(r_t[:], s_t[:])
                            a_t = apool.tile([P, L], r32, tag="a", name=f"a{h}_{i}")
                            nc.vector.tensor_scalar_mul(a_t[:], e_t[:], r_t[:])
                            nc.sync.dma_start(
                                attn_t[h, i * P : (i + 1) * P, :], a_t[:].bitcast(f32)
                            )
                            a_ts[(h, i)] = a_t
                    if i >= AV_LAG:
                        emit_av(i - AV_LAG)
                for h in heads:
                    ho = 64 * (h % 2)
                    nc.vector.tensor_copy(aoT_sb[ho : ho + 64, h // 2, :], po[h][:, :])

        # ---------------- phase 3: output fc ----------------
        with ExitStack() as ph3:
            fcpsum = ph3.enter_context(
                tc.tile_pool(name="fcpsum", bufs=2, space="PSUM")
            )
            fco = ph3.enter_context(tc.tile_pool(name="fco", bufs=3))
            for dt in range(NFT):
                ps = fcpsum.tile([P, L], f32, tag="fc", name=f"fc{dt}")
                for j in range(NQ):
                    sl = slice(j * QC, (j + 1) * QC)
                    for ic in range(NFT):
                        nc.tensor.matmul(
                            ps[:, sl],
                            lhsT=Wf_sb[:, ic, dt * P : (dt + 1) * P],
                            rhs=aoT_sb[:, ic, sl],
                            start=(ic == 0),
                            stop=(ic == NFT - 1),
                        )
                o_t = fco.tile([P, L], f32, tag="fco", name=f"o{dt}")
                nc.vector.tensor_scalar_add(o_t[:], ps[:], bf_sb[:, dt : dt + 1])
                nc.sync.dma_start(outT[dt * P : (dt + 1) * P, :], o_t[:])

    nc.compile()
    return nc


def get_nc():
    if "nc" not in _CACHE:
        _CACHE["nc"] = _build_nc()
    return _CACHE["nc"]


def make_in_maps(q, k, v, mask, Wq, bq, Wk, bk, Wv, bv, Wf, bf):
    bf16 = ml_dtypes.bfloat16
    f32 = np.float32
    q = np.asarray(q, f32)
    k = np.asarray(k, f32)
    v = np.asarray(v, f32)
    mask = np.asarray(mask)
    Wq_, Wk_, Wv_, Wf_ = (np.ascontiguousarray(np.asarray(w, f32)) for w in (Wq, Wk, Wv, Wf))
    bq_ = np.ascontiguousarray(np.asarray(bq, f32).reshape(NFT, P).T)
    bk_ = np.ascontiguousarray(np.asarray(bk, f32).reshape(NFT, P).T)
    bf_ = np.ascontiguousarray(np.asarray(bf, f32).reshape(NFT, P).T)
    bv_ = np.asarray(bv, f32).reshape(1, H * DV).astype(bf16)
    in_maps = []
    for b in range(B):
        maskb = ((np.asarray(mask[b, 0], f32) - 1.0) * 1e9).T
        in_maps.append(
            {
                "xqT": np.ascontiguousarray(q[b].T),
                "xkT": np.ascontiguousarray(k[b].T),
                "xvT": np.ascontiguousarray(v[b].T),
                "maskbT": np.ascontiguousarray(maskb).astype(bf16),
                "Wq": Wq_,
                "Wk": Wk_,
                "Wv": Wv_,
                "Wf": Wf_,
                "bq": bq_,
                "bk": bk_,
                "bv": bv_,
                "bfb": bf_,
            }
        )
    return in_maps


def assemble(results):
    out = np.stack([np.asarray(results[b]["outT"]).T for b in range(B)])
    attn = np.stack(
        [np.asarray(results[b]["attn_t"]).transpose(0, 2, 1) for b in range(B)]
    )
    return out, attn


def kernel(q, k, v, mask, Wq, bq, Wk, bk, Wv, bv, Wf, bf):
    from concourse.bass_utils import run_bass_kernel_spmd

    nc = get_nc()
    in_maps = make_in_maps(q, k, v, mask, Wq, bq, Wk, bk, Wv, bv, Wf, bf)
    res = run_bass_kernel_spmd(nc, in_maps, list(range(B))).results
    return assemble(res)


if __name__ == "__main__":
    nc = get_nc()
    print("built ok")


# revision 14
# speedup vs baseline: 2.0066x; 1.1347x over previous
"""Trainium2 Bass kernel for nn_MultiHeadAttention (softmax over QUERY axis).

Strategy: data-parallel over batch (B=8 -> one batch element per NeuronCore).
Per core (all layouts chosen so every DMA is contiguous and the softmax
reduction runs along the free axis):

  inputs (host pre-transposed):
    xqT/xkT/xvT [D, L] f32, maskbT [L(k), L(q)] bf16 (= (mask-1)*1e9 transposed)
    Wq/Wk/Wv [D, H*DK] f32, Wf [H*DV, D] f32, biases repacked

  QT = Wq^T @ q^T   [HDK, L]  (bf16 stored)      lhsT=Wq chunk, rhs=xqT chunk
  KT = Wk^T @ k^T   [HDK, L]  (bf16 stored)
  V  = v @ Wv       [L, HDV]  (f32)              lhsT=xvT chunk, rhs=Wv chunk
  per head h:
    S_T[k,q] = K_h @ Q_h^T + maskb   (PSUM: identity-matmul copies mask bias
                                      with start=True, QK^T accumulates on top)
    e = exp(S_T/8)  on ScalarE with fused accum_out -> s[k] = sum_q e[k,q]
    attn_T = e * (1/s)   (VectorE tensor_scalar, per-partition scalar)
    out_T[d,q] += V_h^T-as-lhsT @ attn_T         (accumulate over k tiles)
  outT = Wf^T @ aoT + bf   [D, L]

  outputs: attn_t [H, L(k), L(q)] f32, outT [D, L] f32 -- host returns
  transposed views to match the reference layout.
"""

import sys

if "/opt/trn_rl_repo" not in sys.path:
    sys.path.insert(0, "/opt/trn_rl_repo")

import numpy as np
import ml_dtypes

B, L, D = 8, 1024, 1024
H, DK, DV = 16, 64, 64
P = 128
NKT = L // P      # 8 k-tiles
NFT = D // P      # 8 feature/contraction tiles
QC = 512          # matmul moving free-dim chunk (1 PSUM bank of fp32)
NQ = L // QC      # 2

_CACHE = {}


def _build_nc():
    import concourse.bacc as bacc
    import concourse.mybir as mybir
    import concourse.tile as tile
    from concourse.masks import make_identity
    from contextlib import ExitStack

    f32 = mybir.dt.float32
    bf16 = mybir.dt.bfloat16
    r32 = mybir.dt.float32r
    EXP = mybir.ActivationFunctionType.Exp

    nc = bacc.Bacc(None, target_bir_lowering=False, debug=False)

    xqT = nc.dram_tensor("xqT", [D, L], bf16, kind="ExternalInput")
    xkT = nc.dram_tensor("xkT", [D, L], bf16, kind="ExternalInput")
    xvT = nc.dram_tensor("xvT", [D, L], bf16, kind="ExternalInput")
    maskbT = nc.dram_tensor("maskbT", [L, L], bf16, kind="ExternalInput")
    Wq_d = nc.dram_tensor("Wq", [D, H * DK], bf16, kind="ExternalInput")
    Wk_d = nc.dram_tensor("Wk", [D, H * DK], bf16, kind="ExternalInput")
    Wv_d = nc.dram_tensor("Wv", [D, H * DV], bf16, kind="ExternalInput")
    Wf_d = nc.dram_tensor("Wf", [H * DV, D], r32, kind="ExternalInput")
    bq_d = nc.dram_tensor("bq", [P, NFT], f32, kind="ExternalInput")
    bk_d = nc.dram_tensor("bk", [P, NFT], f32, kind="ExternalInput")
    bv_d = nc.dram_tensor("bv", [1, H * DV], bf16, kind="ExternalInput")
    bf_d = nc.dram_tensor("bfb", [P, NFT], f32, kind="ExternalInput")
    attn_t = nc.dram_tensor("attn_t", [H, L, L], f32, kind="ExternalOutput")
    outT = nc.dram_tensor("outT", [D, L], f32, kind="ExternalOutput")

    with tile.TileContext(nc) as tc, ExitStack() as top:
        const = top.enter_context(tc.tile_pool(name="const", bufs=1))
        ident = const.tile([P, P], bf16)
        make_identity(nc, ident[:])
        ones_row = const.tile([1, P], bf16)
        nc.gpsimd.memset(ones_row[:], 1.0)
        bq_sb = const.tile([P, NFT], f32)
        bk_sb = const.tile([P, NFT], f32)
        bf_sb = const.tile([P, NFT], f32)
        bv_sb = const.tile([1, H * DV], bf16)
        nc.sync.dma_start(bq_sb[:], bq_d[:, :])
        nc.sync.dma_start(bk_sb[:], bk_d[:, :])
        nc.sync.dma_start(bf_sb[:], bf_d[:, :])
        nc.sync.dma_start(bv_sb[:], bv_d[:, :])

        maskp = top.enter_context(tc.tile_pool(name="maskp", bufs=1))
        mask_sb = maskp.tile([P, NKT, L], bf16)

        qkv = top.enter_context(tc.tile_pool(name="qkv", bufs=1))
        QT_sb = qkv.tile([P, NFT, L], bf16)
        KT_sb = qkv.tile([P, NFT, L], bf16)
        V_sb = qkv.tile([P, NKT, L], r32)
        aoT_sb = qkv.tile([P, NFT, L], r32)

        # ---------------- phase 1: projections (V first: attention pair 0
        # depends on V fully + QT/KT f-tile 0, so emit its inputs earliest).
        # x and W are bf16: half the DMA bytes, weights fully resident ------
        with ExitStack() as ph1:
            xin = ph1.enter_context(tc.tile_pool(name="xin", bufs=2))
            wres = ph1.enter_context(tc.tile_pool(name="wres", bufs=2))

            with ExitStack() as vph:
                vpsum = vph.enter_context(
                    tc.tile_pool(name="vpsum", bufs=1, space="PSUM")
                )
                xv_sb = xin.tile([P, NFT, L], bf16, tag="xin", name="xv_sb")
                xvr = xvT[:, :].rearrange("(i p) t -> p i t", p=P)
                for ic in range(NFT):
                    nc.scalar.dma_start(xv_sb[:, ic, :], xvr[:, ic, :])
                wv_sb = wres.tile([P, NFT, D], bf16, tag="w", name="wv_sb")
                wvr = Wv_d[:, :].rearrange("(i p) d -> p i d", p=P)
                for ic in range(NFT):
                    nc.sync.dma_start(wv_sb[:, ic, :], wvr[:, ic, :])
                for j in range(NQ):  # halves of the HDV feature range
                    sl = slice(j * QC, (j + 1) * QC)
                    pv = [
                        vpsum.tile([P, QC], f32, tag=f"vps{tt}", name=f"pv{tt}")
                        for tt in range(NKT)
                    ]
                    for ic in range(NFT):
                        for tt in range(NKT):
                            nc.tensor.matmul(
                                pv[tt][:, :],
                                lhsT=xv_sb[:, ic, tt * P : (tt + 1) * P],
                                rhs=wv_sb[:, ic, sl],
                                start=(ic == 0),
                                stop=False,
                            )
                    for tt in range(NKT):
                        # bias add via rank-1 matmul: out[t, f] += 1 * bv[f]
                        nc.tensor.matmul(
                            pv[tt][:, :],
                            lhsT=ones_row[:],
                            rhs=bv_sb[0:1, sl],
                            start=False,
                            stop=True,
                        )
                        nc.vector.tensor_copy(V_sb[:, tt, sl], pv[tt][:, :])

            with ExitStack() as qk:
                ppsum = qk.enter_context(
                    tc.tile_pool(name="ppsum", bufs=1, space="PSUM")
                )
                for xdram, wdram, bias_sb, out_sb in (
                    (xkT, Wk_d, bk_sb, KT_sb),
                    (xqT, Wq_d, bq_sb, QT_sb),
                ):
                    x_sb = xin.tile([P, NFT, L], bf16, tag="xin", name="x_sb")
                    xr = xdram[:, :].rearrange("(i p) t -> p i t", p=P)
                    for ic in range(NFT):
                        nc.scalar.dma_start(x_sb[:, ic, :], xr[:, ic, :])
                    w_sb = wres.tile([P, NFT, D], bf16, tag="w", name="w_sb")
                    wr = wdram[:, :].rearrange("(i p) d -> p i d", p=P)
                    for ic in range(NFT):
                        nc.sync.dma_start(w_sb[:, ic, :], wr[:, ic, :])
                    for fh in range(2):  # halves of the feature tiles
                        fts = range(fh * NFT // 2, (fh + 1) * NFT // 2)
                        ps = {
                            ft: ppsum.tile([P, L], f32, tag=f"pps{ft % 4}", name=f"ps_qk{ft}")
                            for ft in fts
                        }
                        for ic in range(NFT):
                            for ft in fts:
                                for j in range(NQ):
                                    sl = slice(j * QC, (j + 1) * QC)
                                    nc.tensor.matmul(
                                        ps[ft][:, sl],
                                        lhsT=w_sb[:, ic, ft * P : (ft + 1) * P],
                                        rhs=x_sb[:, ic, sl],
                                        start=(ic == 0),
                                        stop=(ic == NFT - 1),
                                    )
                        for ft in fts:
                            nc.vector.tensor_scalar_add(
                                out_sb[:, ft, :], ps[ft][:, :], bias_sb[:, ft : ft + 1]
                            )

        # ---------------- phase 2: attention ----------------
        nc.scalar.dma_start(mask_sb[:], maskbT[:, :].rearrange("(i p) q -> p i q", p=P))
        wfp = top.enter_context(tc.tile_pool(name="wfp", bufs=1))
        Wf_sb = wfp.tile([P, NFT, D], r32)
        wfr = Wf_d[:, :].rearrange("(i p) d -> p i d", p=P)
        for ic in range(NFT):
            nc.scalar.dma_start(Wf_sb[:, ic, :], wfr[:, ic, :])
        with ExitStack() as ph2:
            spsum = ph2.enter_context(tc.tile_pool(name="spsum", bufs=2, space="PSUM"))
            opsum = ph2.enter_context(tc.tile_pool(name="opsum", bufs=2, space="PSUM"))
            epool = ph2.enter_context(tc.tile_pool(name="epool", bufs=3))
            apool = ph2.enter_context(tc.tile_pool(name="apool", bufs=7))
            stat = ph2.enter_context(tc.tile_pool(name="stat", bufs=8))

            AV_LAG = 2  # defer AV matmuls so PE never stalls on the DVE normalize

            for hp in range(H // 2):
                heads = (2 * hp, 2 * hp + 1)
                po = {h: opsum.tile([64, L], f32, tag="po", name=f"po{h}") for h in heads}
                a_ts = {}

                def emit_av(i):
                    for j in range(NQ):
                        sl = slice(j * QC, (j + 1) * QC)
                        for h in heads:
                            nc.tensor.matmul(
                                po[h][:, sl],
                                lhsT=V_sb[:, i, h * DV : (h + 1) * DV],
                                rhs=a_ts[(h, i)][:, sl],
                                start=(i == 0),
                                stop=(i == NKT - 1),
                            )

                for i in range(NKT + AV_LAG):
                    if i < NKT:
                        pss = {}
                        for h in heads:
                            pss[h] = spsum.tile([P, L], f32, tag="ps", name=f"ps{h}_{i}")
                            for j in range(NQ):
                                sl = slice(j * QC, (j + 1) * QC)
                                nc.tensor.matmul(
                                    pss[h][:, sl],
                                    lhsT=ident[:],
                                    rhs=mask_sb[:, i, sl],
                                    start=True,
                                    stop=False,
                                )
                        # S matmuls: alternate heads so adjacent instructions
                        # use disjoint PE row groups (A: rows 0-63, B: 64-127)
                        for j in range(NQ):
                            sl = slice(j * QC, (j + 1) * QC)
                            for h in heads:
                                hi, ho = h // 2, 64 * (h % 2)
                                nc.tensor.matmul(
                                    pss[h][:, sl],
                                    lhsT=KT_sb[ho : ho + 64, hi, i * P : (i + 1) * P],
                                    rhs=QT_sb[ho : ho + 64, hi, sl],
                                    start=False,
                                    stop=True,
                                )
                        for h in heads:
                            e_t = epool.tile([P, L], f32, tag="e", name=f"e{h}_{i}")
                            s_t = stat.tile([P, 1], f32, tag="s", name=f"s{h}_{i}")
                            nc.scalar.activation(
                                e_t[:], pss[h][:, :], EXP, scale=0.125, accum_out=s_t[:]
                            )
                            r_t = stat.tile([P, 1], f32, tag="r", name=f"r{h}_{i}")
                            nc.vector.reciprocal(r_t[:], s_t[:])
                            a_t = apool.tile([P, L], r32, tag="a", name=f"a{h}_{i}")
                            nc.vector.tensor_scalar_mul(a_t[:], e_t[:], r_t[:])
                            nc.sync.dma_start(
                                attn_t[h, i * P : (i + 1) * P, :], a_t[:].bitcast(f32)
                            )
                            a_ts[(h, i)] = a_t
                    if i >= AV_LAG:
                        emit_av(i - AV_LAG)
                for h in heads:
                    ho = 64 * (h % 2)
                    nc.vector.tensor_copy(aoT_sb[ho : ho + 64, h // 2, :], po[h][:, :])

        # ---------------- phase 3: output fc ----------------
        with ExitStack() as ph3:
            fcpsum = ph3.enter_context(
                tc.tile_pool(name="fcpsum", bufs=2, space="PSUM")
            )
            fco = ph3.enter_context(tc.tile_pool(name="fco", bufs=3))
            for dt in range(NFT):
                ps = fcpsum.tile([P, L], f32, tag="fc", name=f"fc{dt}")
                for ic in range(NFT):
                    for j in range(NQ):
                        sl = slice(j * QC, (j + 1) * QC)
                        nc.tensor.matmul(
                            ps[:, sl],
                            lhsT=Wf_sb[:, ic, dt * P : (dt + 1) * P],
                            rhs=aoT_sb[:, ic, sl],
                            start=(ic == 0),
                            stop=(ic == NFT - 1),
                        )
                o_t = fco.tile([P, L], f32, tag="fco", name=f"o{dt}")
                nc.vector.tensor_scalar_add(o_t[:], ps[:], bf_sb[:, dt : dt + 1])
                nc.sync.dma_start(outT[dt * P : (dt + 1) * P, :], o_t[:])

    nc.compile()
    return nc


def get_nc():
    if "nc" not in _CACHE:
        _CACHE["nc"] = _build_nc()
    return _CACHE["nc"]


def make_in_maps(q, k, v, mask, Wq, bq, Wk, bk, Wv, bv, Wf, bf):
    bf16 = ml_dtypes.bfloat16
    f32 = np.float32
    q = np.asarray(q, f32)
    k = np.asarray(k, f32)
    v = np.asarray(v, f32)
    mask = np.asarray(mask)
    Wq_, Wk_, Wv_ = (np.ascontiguousarray(np.asarray(w, f32)).astype(bf16) for w in (Wq, Wk, Wv))
    Wf_ = np.ascontiguousarray(np.asarray(Wf, f32))
    bq_ = np.ascontiguousarray(np.asarray(bq, f32).reshape(NFT, P).T)
    bk_ = np.ascontiguousarray(np.asarray(bk, f32).reshape(NFT, P).T)
    bf_ = np.ascontiguousarray(np.asarray(bf, f32).reshape(NFT, P).T)
    bv_ = np.asarray(bv, f32).reshape(1, H * DV).astype(bf16)
    in_maps = []
    for b in range(B):
        maskb = ((np.asarray(mask[b, 0], f32) - 1.0) * 1e9).T
        in_maps.append(
            {
                "xqT": np.ascontiguousarray(q[b].T).astype(bf16),
                "xkT": np.ascontiguousarray(k[b].T).astype(bf16),
                "xvT": np.ascontiguousarray(v[b].T).astype(bf16),
                "maskbT": np.ascontiguousarray(maskb).astype(bf16),
                "Wq": Wq_,
                "Wk": Wk_,
                "Wv": Wv_,
                "Wf": Wf_,
                "bq": bq_,
                "bk": bk_,
                "bv": bv_,
                "bfb": bf_,
            }
        )
    return in_maps


def assemble(results):
    out = np.stack([np.asarray(results[b]["outT"]).T for b in range(B)])
    attn = np.stack(
        [np.asarray(results[b]["attn_t"]).transpose(0, 2, 1) for b in range(B)]
    )
    return out, attn


def kernel(q, k, v, mask, Wq, bq, Wk, bk, Wv, bv, Wf, bf):
    from concourse.bass_utils import run_bass_kernel_spmd

    nc = get_nc()
    in_maps = make_in_maps(q, k, v, mask, Wq, bq, Wk, bk, Wv, bv, Wf, bf)
    res = run_bass_kernel_spmd(nc, in_maps, list(range(B))).results
    return assemble(res)


if __name__ == "__main__":
    nc = get_nc()
    print("built ok")
